# revision 19
# baseline (speedup 1.0000x reference)
"""MixedFFN Trainium2 kernel (8 NeuronCores, SPMD) — v2.

Problem: x [8, 2048, 1024]; shared FFN (W1S [2048,1024], W2S [1024,2048])
applied to positions 0..1984 of every batch; per-position FFN
(W1NS [64,1024,2048], W2NS [64,2048,1024]) applied to positions 1984..2048.
gelu is exact (erf). Output [8, 2048, 1024] fp32.

Sharding:
  - Shared part: data-parallel over batch. Core i computes the shared FFN
    for batch i (rows 0..1984; the last row-block streams only 448 cols).
  - Per-position part: sharded over positions. Core i handles positions
    1984+8i .. 1984+8(i+1) for ALL batches (two groups of 4 positions).

v2 changes vs v1 (561us):
  - All matmul operands fp16 (PE streams at 1 col/cycle for f32r and fp16
    alike, but fp16 halves HBM traffic: ~128MB -> ~81MB per core).
  - NS matmuls col-tiled: 4 positions ride the PE's four 32-column tiles
    concurrently (tile_position=(0,32j)), cutting NS PE time ~4x.
  - NS h transposes moved off the PE onto the DMA XBAR (SBUF->SBUF
    dma_start(transpose=True), 16-bit).
  - Outputs written fp16 (host upcasts); YS/YN + transposes ride the ACT
    HWDGE ring, bulk weight streams the SP ring.
  - PSUM: shared MM1 2 banks + shared MM2 4 banks + NS 2 banks = 8.
"""

import os
import sys

import numpy as np

for _p in ("/opt/trn_rl_repo",):
    if os.path.isdir(_p) and _p not in sys.path:
        sys.path.insert(0, _p)

B, T, D, F, LNS = 8, 2048, 1024, 2048, 64
S = T - LNS  # 1984
NCORES = 8
PPC = LNS // NCORES  # 8 positions per core
NG = 2  # groups of 4 positions
FB = 4  # f-blocks of 512 for NS MM1
RB = 512  # shared row-block
NRB = T // RB  # 4
DC, FC = D // 128, F // 128  # 8, 16 k-chunks

LAST_RESULTS = None  # BassKernelResults of the most recent run (for test.py)

_cached = None


def _interleave(a, b):
    """Merge two step lists proportionally (Bresenham); each step is a
    zero-arg callable that emits instructions."""
    if not b:
        return list(a)
    if not a:
        return list(b)
    out = []
    ia = ib = 0
    na, nb = len(a), len(b)
    while ia < na or ib < nb:
        if ib * na <= ia * nb:
            if ib < nb:
                out.append(b[ib])
                ib += 1
            else:
                out.append(a[ia])
                ia += 1
        else:
            if ia < na:
                out.append(a[ia])
                ia += 1
            else:
                out.append(b[ib])
                ib += 1
    return out


def _clump(steps, k):
    """Group consecutive steps into single callables of k steps each, so
    interleaving with shared work produces few PE tile-mode switches."""
    out = []
    for i in range(0, len(steps), k):
        chunk = steps[i : i + k]

        def run(chunk=chunk):
            for st in chunk:
                st()

        out.append(run)
    return out


def _build():
    import concourse.tile as tile
    from concourse import bacc
    from concourse import mybir

    f32 = mybir.dt.float32
    f16 = mybir.dt.float16
    GELU = (
        mybir.ActivationFunctionType.Relu
        if os.environ.get("MIXEDFFN_SIM_ACT") == "relu"
        else mybir.ActivationFunctionType.Gelu
    )

    nc = bacc.Bacc("TRN2", target_bir_lowering=False, debug=False, num_devices=NCORES)

    XT = nc.dram_tensor("XT", [NRB, 128, DC, RB], f16, kind="ExternalInput").ap()
    XNS = nc.dram_tensor("XNS", [128, DC, NG, 4, 16], f16, kind="ExternalInput").ap()
    W1T = nc.dram_tensor("W1T", [FC, 128, DC, 128], f16, kind="ExternalInput").ap()
    W2T = nc.dram_tensor("W2T", [F, D], f16, kind="ExternalInput").ap()
    W1N = nc.dram_tensor("W1N", [NG, FB, 4, 128, 4096], f16, kind="ExternalInput").ap()
    W2N = nc.dram_tensor("W2N", [NG, 2, 8, 128, 4096], f16, kind="ExternalInput").ap()
    IDEN = nc.dram_tensor("IDEN", [128, 128], f16, kind="ExternalInput").ap()
    YS = nc.dram_tensor("YS", [S, D], f16, kind="ExternalOutput").ap()
    YN = nc.dram_tensor("YN", [NG, 2, 128, 512], f16, kind="ExternalOutput").ap()

    with tile.TileContext(nc) as tc:
        with (
            tc.tile_pool(name="wres", bufs=1) as wres,
            tc.tile_pool(name="xt", bufs=2) as xtp,
            tc.tile_pool(name="hth", bufs=1) as hthp,
            tc.tile_pool(name="ht01", bufs=2) as ht01p,
            tc.tile_pool(name="ysb", bufs=2) as ysbp,
            tc.tile_pool(name="w1n", bufs=5) as w1np,
            tc.tile_pool(name="w2n", bufs=5) as w2np,
            tc.tile_pool(name="hns", bufs=4) as hnsp,
            tc.tile_pool(name="htns", bufs=4) as htnsp,
            tc.tile_pool(name="ynsb", bufs=2) as ynsbp,
            tc.tile_pool(name="ph", bufs=2, space="PSUM") as php,
            tc.tile_pool(name="py", bufs=2, space="PSUM") as pyp,
            tc.tile_pool(name="nsps", bufs=2, space="PSUM") as nsps,
        ):
            # ---- resident tiles; shared weight loads are emitted as
            # interleaved steps so the first matmuls start early ----
            warm = wres.tile([128, 512], f16)
            nc.vector.memset(warm[:], 0.0)
            wps = php.tile([128, 512], f32, name="warmps", tag="ph")
            for _ in range(20):
                nc.tensor.matmul(
                    wps[:], warm[:, 0:128], warm[:], start=True, stop=True,
                    skip_group_check=True,
                )
            w1t_sb = wres.tile([128, FC, DC, 128], f16)
            w2t_sb = wres.tile([128, FC, D], f16)
            xns_sb = wres.tile([128, DC, NG, 4, 16], f16)
            nc.scalar.dma_start(out=xns_sb[:], in_=XNS[:])
            ident = wres.tile([128, 128], f16)
            nc.scalar.dma_start(out=ident[:], in_=IDEN[:])

            def w_load_steps():
                def w1_step(fc):
                    def step():
                        nc.sync.dma_start(out=w1t_sb[:, fc], in_=W1T[fc])

                    return step

                def w2_step(fc):
                    def step():
                        nc.sync.dma_start(
                            out=w2t_sb[:, fc, :],
                            in_=W2T[fc * 128 : (fc + 1) * 128, :],
                        )

                    return step

                out = []
                for fc in range(FC):
                    out.append(w1_step(fc))
                    out.append(w2_step(fc))
                return out

            # ---- step generators ----
            state = {}

            def xt_load(rb):
                def step():
                    xt = xtp.tile([128, DC, RB], f16, name=f"xtt{rb}", tag="xtt")
                    nc.sync.dma_start(out=xt[:], in_=XT[rb])
                    state[("xt", rb)] = xt

                return step

            def shared_steps(rb):
                R = 448 if rb == NRB - 1 else 512  # trim discarded tail rows
                steps = [xt_load(rb)] if rb == 0 else []

                def fc_step(fc):
                    def step():
                        xt = state[("xt", rb)]
                        if fc == 0:
                            state[("hth", rb)] = hthp.tile(
                                [128, FC, 256], f16, name=f"hth{rb}", tag="hth"
                            )
                            state["py0"] = pyp.tile(
                                [128, D], f32, name=f"py{rb}_0", tag="py"
                            )
                            state["py1"] = pyp.tile(
                                [128, D], f32, name=f"py{rb}_1", tag="py"
                            )
                        hth = state[("hth", rb)]
                        ph = php.tile([128, RB], f32, name=f"ph{rb}_{fc}", tag="ph")
                        for dc in range(DC):
                            nc.tensor.matmul(
                                ph[:, 0:R],
                                w1t_sb[:, fc, dc, :],
                                xt[:, dc, 0:R],
                                start=(dc == 0),
                                stop=(dc == DC - 1),
                                skip_group_check=True,
                            )
                        ht01 = ht01p.tile(
                            [128, 256], f16, name=f"ht01_{rb}_{fc}", tag="ht01"
                        )
                        nc.scalar.activation(ht01[:], ph[:, 0:256], GELU)
                        nc.scalar.activation(
                            hth[:, fc, 0 : R - 256], ph[:, 256:R], GELU
                        )
                        for rc in range(2):
                            py = state[f"py{rc}"]
                            for dh in range(2):
                                nc.tensor.matmul(
                                    py[:, dh * 512 : (dh + 1) * 512],
                                    ht01[:, rc * 128 : (rc + 1) * 128],
                                    w2t_sb[:, fc, dh * 512 : (dh + 1) * 512],
                                    start=(fc == 0),
                                    stop=(fc == FC - 1),
                                    skip_group_check=True,
                                )

                    return step

                def y01_step(rc):
                    def step():
                        py = state[f"py{rc}"]
                        ysb = ysbp.tile(
                            [128, D], f16, name=f"ysb{rb}_{rc}", tag="ysb"
                        )
                        nc.vector.tensor_copy(ysb[:], py[:])
                        row0 = rb * RB + rc * 128
                        nc.sync.dma_start(
                            out=YS[row0 : row0 + 128, :], in_=ysb[:, :]
                        )

                    return step

                def mm2b_step(rc):
                    def step():
                        hth = state[("hth", rb)]
                        nrows = min(128, S - (rb * RB + rc * 128))
                        py = pyp.tile([128, D], f32, name=f"py{rb}_{rc}", tag="py")
                        for dh in range(2):
                            for fc in range(FC):
                                nc.tensor.matmul(
                                    py[0:nrows, dh * 512 : (dh + 1) * 512],
                                    hth[
                                        :,
                                        fc,
                                        (rc - 2) * 128 : (rc - 2) * 128 + nrows,
                                    ],
                                    w2t_sb[:, fc, dh * 512 : (dh + 1) * 512],
                                    start=(fc == 0),
                                    stop=(fc == FC - 1),
                                    skip_group_check=True,
                                )
                        ysb = ysbp.tile(
                            [128, D], f16, name=f"ysb{rb}_{rc}", tag="ysb"
                        )
                        nc.vector.tensor_copy(ysb[0:nrows, :], py[0:nrows, :])
                        row0 = rb * RB + rc * 128
                        nc.sync.dma_start(
                            out=YS[row0 : row0 + nrows, :], in_=ysb[0:nrows, :]
                        )

                    return step

                for fc in range(FC):
                    steps.append(fc_step(fc))
                    if fc == 7 and rb + 1 < NRB:
                        steps.append(xt_load(rb + 1))
                steps += [y01_step(0), y01_step(1), mm2b_step(2), mm2b_step(3)]
                return steps

            def ns_steps(g):
                """NS 4-position group: per fb 8 MM1 (dc) + 1 transpose step,
                then 32 MM2 (dh,fc). NS matmuls are col-tiled: position j
                rides PE columns 32j..32j+32; each [128,128] PE transpose
                covers all 4 positions at once. Weight DMAs (1MB, two
                k-chunks each) are separate steps emitted ~6 MM-steps ahead
                of their consumers and alternate between the two HWDGE
                rings so the issue stream keeps up with the PE."""

                def w1_dma(fb, dcp, eng):
                    def step():
                        w1 = w1np.tile(
                            [128, 2, 4, 512], f16, name=f"w1_{g}_{fb}_{dcp}", tag="w1"
                        )
                        state[("w1", fb, dcp)] = w1
                        eng.dma_start(out=w1[:], in_=W1N[g, fb, dcp])

                    return step

                def w2_dma(dh, fcp, eng):
                    def step():
                        w2 = w2np.tile(
                            [128, 2, 4, 512], f16, name=f"w2_{g}_{dh}_{fcp}", tag="w2"
                        )
                        state[("w2", dh, fcp)] = w2
                        eng.dma_start(out=w2[:], in_=W2N[g, dh, fcp])

                    return step

                def mm1_step(fb, dc):
                    def step():
                        if dc == 0:
                            state["phn"] = nsps.tile(
                                [128, 512], f32, name=f"phn{g}_{fb}", tag="nsps"
                            )
                        phn = state["phn"]
                        w1 = state[("w1", fb, dc // 2)]
                        for j in range(4):
                            nc.tensor.matmul(
                                phn[32 * j : 32 * j + B, :],
                                xns_sb[:, dc, g, j, 0:B],
                                w1[:, dc % 2, j, :],
                                start=(dc == 0),
                                stop=(dc == DC - 1),
                                skip_group_check=True,
                                tile_position=(0, 32 * j),
                            )
                        if dc == DC - 1:
                            hsb = hnsp.tile(
                                [128, 512], f16, name=f"hsb{g}_{fb}", tag="hsb"
                            )
                            state[("hsb", fb)] = hsb
                            nc.scalar.activation(hsb[:], phn[:], GELU)

                    return step

                def tr_step(fb):
                    def step():
                        hsb = state[("hsb", fb)]
                        pt4 = nsps.tile(
                            [128, 4, 128], f16, name=f"pt{g}_{fb}", tag="nsps"
                        )
                        for k in range(4):
                            nc.tensor.transpose(
                                pt4[:, k, :],
                                hsb[:, k * 128 : (k + 1) * 128],
                                ident[:],
                            )
                        hT = htnsp.tile(
                            [128, 4, 128], f16, name=f"hT{g}_{fb}", tag="hT"
                        )
                        state[("hT", fb)] = hT
                        nc.vector.tensor_copy(hT[:], pt4[:])

                    return step

                def mm2_step(dh, fc):
                    def step():
                        if fc == 0:
                            state["pyn"] = nsps.tile(
                                [128, 512], f32, name=f"pyn{g}_{dh}", tag="nsps"
                            )
                        pyn = state["pyn"]
                        hT = state[("hT", fc // 4)]
                        w2 = state[("w2", dh, fc // 2)]
                        for j in range(4):
                            nc.tensor.matmul(
                                pyn[32 * j : 32 * j + B, :],
                                hT[:, fc % 4, 32 * j : 32 * j + B],
                                w2[:, fc % 2, j, :],
                                start=(fc == 0),
                                stop=(fc == FC - 1),
                                skip_group_check=True,
                                tile_position=(0, 32 * j),
                            )
                        if fc == FC - 1:
                            ynsb = ynsbp.tile(
                                [128, 512], f16, name=f"ynsb{g}_{dh}", tag="ynsb"
                            )
                            nc.vector.tensor_copy(ynsb[:], pyn[:])
                            nc.sync.dma_start(out=YN[g, dh], in_=ynsb[:])

                    return step

                engines = [nc.sync, nc.scalar]
                dmas = [
                    w1_dma(fb, dcp, engines[(fb * 4 + dcp) % 2])
                    for fb in range(FB)
                    for dcp in range(4)
                ] + [
                    w2_dma(dh, fcp, engines[(dh * 8 + fcp) % 2])
                    for dh in range(2)
                    for fcp in range(8)
                ]
                mms = []
                for fb in range(FB):
                    mms += [mm1_step(fb, dc) for dc in range(DC)]
                    if fb >= 1:
                        # tr(fb-1) after mm1(fb): its gelu overlapped fb's chain
                        mms.append(tr_step(fb - 1))
                mms.append(tr_step(FB - 1))
                mms += [mm2_step(dh, fc) for dh in range(2) for fc in range(FC)]
                # weave: 3 DMAs of lead, then Bresenham the remaining 29
                # across the 68 MM steps
                return dmas[0:3] + _interleave(mms, dmas[3:])

            # ---- emission: weave W loads through rb0 (program order IS the
            # dependency order under Tile: a consumer emitted before its
            # producer reads stale data). NS clumps are spread across later
            # row-blocks where shared DMA is light. ----
            all_ns = [st for g in range(NG) for st in ns_steps(g)]
            clumps = _clump(all_ns, 4)  # 50 clumps
            nsplit = [4, 15, 16, 15]
            assert sum(nsplit) == len(clumps), (len(clumps), nsplit)
            ns_off = [0]
            for c in nsplit:
                ns_off.append(ns_off[-1] + c)
            for rb in range(NRB):
                sh = shared_steps(rb)
                if rb == 0:
                    wl = w_load_steps()  # [w1(0), w2(0), w1(1), w2(1), ...]
                    woven = [sh[0]] + wl[0:4]
                    rest = sh[1:]
                    for k, st in enumerate(rest):
                        woven.append(st)
                        lo, hi = 4 + 2 * k, 4 + 2 * (k + 1)
                        woven += wl[lo:hi]
                    sh = woven
                nsl = clumps[ns_off[rb] : ns_off[rb + 1]]
                if rb == 0:
                    seq = sh[:10] + _interleave(sh[10:], nsl)
                elif rb == NRB - 1:
                    seq = _interleave(sh[:-3], nsl) + sh[-3:]
                else:
                    seq = _interleave(sh, nsl)
                for st in seq:
                    st()

    nc.compile()
    return nc


def _prepare_inputs(x, W1S, W2S, W1NS, W2NS):
    x = np.ascontiguousarray(x, dtype=np.float32)
    # [FC, 128, DC, 128] per-fc contiguous blocks of W1S.T
    w1t = np.ascontiguousarray(
        np.asarray(W1S.T, dtype=np.float32)
        .reshape(DC, 128, FC, 128)
        .transpose(2, 1, 0, 3)
        .astype(np.float16)
    )
    w2t = np.ascontiguousarray(W2S.T.astype(np.float16))  # [F, D]
    in_maps = []
    for i in range(NCORES):
        # [NRB, 128, DC, RB]: per row-block, partition-major
        xt = np.ascontiguousarray(
            x[i].T.reshape(DC, 128, NRB, RB).transpose(2, 1, 0, 3).astype(np.float16)
        )
        xi = x[:, S + PPC * i : S + PPC * (i + 1), :]  # [B, 8, D]
        # [128, DC, NG, 4, 16] (batch padded 8->16 for 32B-aligned slices)
        xns4 = (
            xi.transpose(2, 1, 0)  # [D, 8, B]
            .reshape(DC, 128, NG, 4, B)
            .transpose(1, 0, 2, 3, 4)
            .astype(np.float16)
        )  # [128, DC, NG, 4, B]
        xns = np.zeros((128, DC, NG, 4, 16), dtype=np.float16)
        xns[..., :B] = xns4
        # W1N [NG, FB, 4, 128, 2*4*512]: [g, fb, dcp, r, half*2048+j*512+c]
        w1n = (
            W1NS[PPC * i : PPC * (i + 1)]
            .astype(np.float16)
            .reshape(NG, 4, 4, 2, 128, FB, 512)  # [g, j, dcp, half, r, fb, c]
            .transpose(0, 5, 2, 4, 3, 1, 6)
            .reshape(NG, FB, 4, 128, 4096)
        )
        # W2N [NG, 2, 8, 128, 2*4*512]: [g, dh, fcp, r, half*2048+j*512+d']
        w2n = (
            W2NS[PPC * i : PPC * (i + 1)]
            .astype(np.float16)
            .reshape(NG, 4, 8, 2, 128, 2, 512)  # [g, j, fcp, half, r, dh, d']
            .transpose(0, 5, 2, 4, 3, 1, 6)
            .reshape(NG, 2, 8, 128, 4096)
        )
        in_maps.append(
            {
                "XT": xt,
                "XNS": np.ascontiguousarray(xns),
                "W1T": w1t,
                "W2T": w2t,
                "W1N": np.ascontiguousarray(w1n),
                "W2N": np.ascontiguousarray(w2n),
                "IDEN": np.eye(128, dtype=np.float16),
            }
        )
    return in_maps


def kernel(x, W1S, W2S, W1NS, W2NS):
    global _cached, LAST_RESULTS
    from concourse.bass_utils import run_bass_kernel_spmd

    if _cached is None:
        _cached = _build()
    nc = _cached
    in_maps = _prepare_inputs(x, W1S, W2S, W1NS, W2NS)
    trace = bool(os.environ.get("MIXEDFFN_TRACE"))
    res = run_bass_kernel_spmd(
        nc, in_maps, core_ids=list(range(NCORES)), trace=trace
    )
    LAST_RESULTS = res
    out = np.empty((B, T, D), dtype=np.float32)
    for i in range(NCORES):
        out[i, :S, :] = res.results[i]["YS"].astype(np.float32)
        yn = res.results[i]["YN"].astype(np.float32)  # [NG, 2, 128, 512]
        # [g, dh, 32j+b, d'] -> out[b, S+8i+4g+j, dh*512+d']
        yn = yn.reshape(NG, 2, 4, 32, 512)[:, :, :, :B, :]
        yn = yn.transpose(3, 0, 2, 1, 4).reshape(B, PPC, D)
        out[:, S + PPC * i : S + PPC * (i + 1), :] = yn
    return out
